# revision 22
# baseline (speedup 1.0000x reference)
"""KAN block (RBF-spline einsum) Trainium2 kernel.

Computes out[b,o] = sum_{i,k} W[o,i,k] * exp(-0.5*((x[b,i]-knots[k])/h)^2)
for B=2048, IN=1024, OUT=1024, K=20 on 8 NeuronCores.

Strategy
--------
Sharding: 4-way over batch x 2-way over out_features (pure-concat gather).
This splits the (unsharded-by-out) basis computation 4 ways so the scalar
and vector engines stay well under the tensor-engine time, which is the
roofline: 2*B*OUT*IN*K/8 = 10.7 GFLOP/core at ~78 TF/s bf16 ~= 137 us.

Math: on a uniform knot grid t_k = t_0 + k*h the basis factorizes:
    basis_k = exp(-((x-t_a)^2)/(2h^2)) * exp(j*x/h) * g_k,   k = a + j
with g_k = exp(-j*t_a/h - j^2/2) a per-k constant folded into W on the host.
Per 128-row i-tile the device computes 5 "anchor" Gaussians on the scalar
engine (Square then Exp, constants folded into the activation's scale/bias)
plus exp(x/h), exp(-x/h), exp(2x/h), and fills the remaining 15 of 20 knot
slices with single bf16 vector multiplies (j in {-1,+1,+2} around each
anchor). The tensor engine contracts (i,k) against the pre-transposed,
pre-scaled W shard with 80 accumulating bf16 matmuls per i-tile into PSUM.
Anchor chains span at most 2h, so intermediate magnitudes stay well inside
bf16/fp32 range for any plausible x (the naive single-chain recurrence
overflows/underflows catastrophically).
"""

import math
import sys

import numpy as np

for _p in ("/opt/trn_rl_repo",):
    if _p not in sys.path:
        sys.path.append(_p)

import ml_dtypes

import concourse.bass as bass
import concourse.tile as tile
from concourse import bacc, mybir
from concourse import bass_utils

F32 = mybir.dt.float32
BF16 = mybir.dt.bfloat16
AF = mybir.ActivationFunctionType

B, IN, OUT, K = 2048, 1024, 1024, 20
N_CORES = 8
B_SHARDS, O_SHARDS = 2, 4
BC = B // B_SHARDS          # 1024 batch rows per core
OC = OUT // O_SHARDS        # 256 out features per core
N_ITILES = IN // 128        # 8
N_OTILES = OC // 128        # 2
N_BTILES = BC // 512        # 2 (PSUM-bank-sized batch halves)
ANCHORS = (1, 5, 9, 13, 17)  # anchor knot indices; offsets j in {-1,0,1,2}
# matmul consumption order: segment by segment, anchor first within each —
# matches production order so the PE never waits long; W's k-axis is stored
# in this order on the host
KORDER = [k for a in ANCHORS for k in (a, a - 1, a + 1, a + 2)]
WARMUP_MM = 17

_cache: dict = {}


def _build_program(h: float, t0: float, reps: int = 1):
    """Build + compile the single-core Bass program (same for all cores)."""
    nc = bacc.Bacc(
        "TRN2",
        target_bir_lowering=False,
        debug=False,
        enable_asserts=False,
        num_devices=N_CORES,
    )
    xt_d = nc.dram_tensor("xt", [IN, BC], F32, kind="ExternalInput")
    wt_d = nc.dram_tensor("wt", [N_ITILES, 128, K, OC], BF16, kind="ExternalInput")
    out_d = nc.dram_tensor("out", [OC, BC], F32, kind="ExternalOutput")
    xt, wt, out = xt_d.ap(), wt_d.ap(), out_d.ap()

    inv_h = 1.0 / h
    s2h = 1.0 / (math.sqrt(2.0) * h)

    korder = KORDER

    with tile.TileContext(nc) as tc:
        with (
            tc.tile_pool(name="xp", bufs=2) as xp,
            tc.tile_pool(name="wp", bufs=3) as wp,
            tc.tile_pool(name="rp", bufs=2) as rp,
            tc.tile_pool(name="sp", bufs=3) as sp,
            tc.tile_pool(name="bp", bufs=2) as bp,
            tc.tile_pool(name="op", bufs=4) as op,
            tc.tile_pool(name="cp", bufs=1) as cp,
            tc.tile_pool(name="ps", bufs=1, space=bass.MemorySpace.PSUM) as ps,
        ):
            sq_bias = []
            for s, a in enumerate(ANCHORS):
                ta = t0 + a * h
                bt = cp.tile([128, 1], F32, tag=f"bias{s}", name=f"bias{s}")
                nc.gpsimd.memset(bt[:], -ta * s2h)
                sq_bias.append(bt)
            # dummy activation: pulls the exp_and_others table load into the
            # constant-setup region so it overlaps the first input DMAs
            # instead of gating the first real Square/Exp
            warm = cp.tile([128, 1], F32, tag="warm", name="warm")
            nc.scalar.activation(warm[:], sq_bias[0][:], AF.Exp, scale=0.0)
            # dummy matmul chain: ~3.5us of PE activity during the DMA fill
            # trips the HAM clock gate to 8/8 so the real matmuls start warm
            if WARMUP_MM:
                wsc = cp.tile([128, 512], BF16, tag="wsc", name="wsc")
                nc.gpsimd.memset(wsc[:], 1.0)
                ps_w = ps.tile([128, 512], F32, tag="psw", name="psw")
                for _w in range(WARMUP_MM):
                    nc.tensor.matmul(ps_w[:], wsc[:, 0:128], wsc[:],
                                     start=True, stop=True)

            def body(_=None):
                psum = [
                    ps.tile([128, 512], F32, tag=f"ps{u}", name=f"ps{u}")
                    for u in range(N_OTILES * N_BTILES)
                ]
                for it in range(N_ITILES):
                    x_t = xp.tile([128, BC], F32, tag="x", name="x_t")
                    nc.sync.dma_start(x_t[:], xt[it * 128:(it + 1) * 128, :])
                    # W's k-axis is host-permuted to KORDER; split the DMA so
                    # the first segment's slices land first and the PE can
                    # start early.
                    w_t = wp.tile([128, K, OC], BF16, tag="w", name="w_t")
                    nc.sync.dma_start(w_t[:, 0:4, :], wt[it, :, 0:4, :])
                    nc.sync.dma_start(w_t[:, 4:K, :], wt[it, :, 4:K, :])

                    basis = bp.tile([128, K, BC], BF16, tag="basis", name="basis")

                    # first anchor ASAP, then the r powers, then the rest
                    sq0 = sp.tile([128, BC], F32, tag="sq", name="sq0")
                    nc.scalar.activation(
                        sq0[:], x_t[:], AF.Square, scale=s2h, bias=sq_bias[0][:]
                    )
                    a0 = ANCHORS[0]
                    nc.scalar.activation(basis[:, a0, :], sq0[:], AF.Exp, scale=-1.0)
                    rb = rp.tile([128, BC], BF16, tag="rb", name="rb")
                    nc.scalar.activation(rb[:], x_t[:], AF.Exp, scale=inv_h)
                    rib = rp.tile([128, BC], BF16, tag="rib", name="rib")
                    nc.scalar.activation(rib[:], x_t[:], AF.Exp, scale=-inv_h)
                    r2b = rp.tile([128, BC], BF16, tag="r2b", name="r2b")
                    nc.scalar.activation(r2b[:], x_t[:], AF.Exp, scale=2.0 * inv_h)
                    for s, a in enumerate(ANCHORS):
                        if s > 0:
                            sq = sp.tile([128, BC], F32, tag="sq", name="sq")
                            nc.scalar.activation(
                                sq[:], x_t[:], AF.Square, scale=s2h, bias=sq_bias[s][:]
                            )
                            nc.scalar.activation(
                                basis[:, a, :], sq[:], AF.Exp, scale=-1.0
                            )
                        nc.vector.tensor_mul(basis[:, a - 1, :], basis[:, a, :], rib[:])
                        nc.vector.tensor_mul(basis[:, a + 1, :], basis[:, a, :], rb[:])
                        nc.vector.tensor_mul(basis[:, a + 2, :], basis[:, a, :], r2b[:])

                    # bb innermost: consecutive matmuls share the stationary
                    # W tile, which measures ~12 ns/MM faster than a fresh
                    # Ldweights per matmul
                    if it < N_ITILES - 1:
                        for j, k in enumerate(korder):
                            for ot in range(N_OTILES):
                                for bb in range(N_BTILES):
                                    nc.tensor.matmul(
                                        psum[ot * N_BTILES + bb][:],
                                        w_t[:, j, ot * 128:(ot + 1) * 128],
                                        basis[:, k, bb * 512:(bb + 1) * 512],
                                        start=(it == 0 and j == 0),
                                        stop=False,
                                    )
                    else:
                        # last i-tile: finish one psum bank at a time so its
                        # copy-out + DMA overlap the remaining matmuls
                        for ot in range(N_OTILES):
                            for bb in range(N_BTILES):
                                u = ot * N_BTILES + bb
                                for j, k in enumerate(korder):
                                    nc.tensor.matmul(
                                        psum[u][:],
                                        w_t[:, j, ot * 128:(ot + 1) * 128],
                                        basis[:, k, bb * 512:(bb + 1) * 512],
                                        start=False,
                                        stop=(j == K - 1),
                                    )
                                o_t = op.tile([128, 512], F32, tag=f"o{u}",
                                              name=f"o_t{u}")
                                nc.scalar.copy(o_t[:], psum[u][:])
                                nc.sync.dma_start(
                                    out[ot * 128:(ot + 1) * 128,
                                        bb * 512:(bb + 1) * 512],
                                    o_t[:],
                                )

            if reps == 1:
                body()
            else:
                with tc.For_i(0, reps, 1) as _i:
                    body(_i)

    nc.compile()
    return nc


def _get_program(h: float, t0: float, reps: int = 1):
    key = (round(h, 9), round(t0, 9), reps)
    if key not in _cache:
        _cache[key] = _build_program(h, t0, reps)
    return _cache[key]


def _prep_inputs(x, W, knots):
    """Host-side sharding/layout. Returns in_maps for the 8 cores."""
    x = np.asarray(x, dtype=np.float32)
    W = np.asarray(W, dtype=np.float32)
    knots = np.asarray(knots, dtype=np.float64)
    h = float(knots[1] - knots[0])
    t0 = float(knots[0])

    # fold the per-knot constants g_k = exp(-j*t_a/h - j^2/2) into W
    g = np.empty(K, dtype=np.float64)
    for s, a in enumerate(ANCHORS):
        ta = knots[a]
        for j in (-1, 0, 1, 2):
            g[a + j] = math.exp(-j * ta / h - 0.5 * j * j)
    Wg = W.astype(np.float64) * g[None, None, :]

    # per o-shard: [i_tile, i_in_tile, k(korder), o] contiguous per i-tile, bf16
    wts = []
    for os_ in range(O_SHARDS):
        wc = Wg[os_ * OC:(os_ + 1) * OC, :, KORDER]            # (OC, IN, K)
        wt = np.ascontiguousarray(wc.transpose(1, 2, 0).reshape(N_ITILES, 128, K, OC))
        wts.append(wt.astype(ml_dtypes.bfloat16))
    xts = []
    for bs in range(B_SHARDS):
        xts.append(np.ascontiguousarray(x[bs * BC:(bs + 1) * BC].T))  # (IN, BC)

    in_maps = []
    for c in range(N_CORES):
        bs, os_ = divmod(c, O_SHARDS)
        in_maps.append({"xt": xts[bs], "wt": wts[os_]})
    return in_maps, h, t0


def kernel(x, W, knots):
    assert x.shape == (B, IN) and W.shape == (OUT, IN, K) and knots.shape == (K,)
    in_maps, h, t0 = _prep_inputs(x, W, knots)
    nc = _get_program(h, t0, reps=1)
    res = bass_utils.run_bass_kernel_spmd(nc, in_maps, core_ids=list(range(N_CORES)))
    out = np.empty((B, OUT), dtype=np.float32)
    for c in range(N_CORES):
        bs, os_ = divmod(c, O_SHARDS)
        shard = res.results[c]["out"]  # (OC, BC) [o, b]
        out[bs * BC:(bs + 1) * BC, os_ * OC:(os_ + 1) * OC] = shard.T
    return out
